# revision 2
# baseline (speedup 1.0000x reference)
"""PhasorTransformer kernel for 8x TRN2 NeuronCores (v2: int8 phases).

Math: the reference applies, per batch row b, 4 blocks of
(diag phase shift -> ortho DFT -> diag phase shift) to z0 = exp(i*x[b,:]),
then reads out asin(sin(angle(z[:, 0]))).  Everything after z0 is linear in
z0, so z_final[b, 0] = <z0[b, :], v> for a fixed complex vector v ("column 0"
of the composed operator) that depends only on the weights.  With
v[t] = m[t] * exp(i*phi[t]):

    re[b] = sum_t m[t] * cos(x[b,t] + phi[t])
    im[b] = sum_t m[t] * sin(x[b,t] + phi[t])
    out[b] = atan-fold(im / |re|) * sign(im)

v2 pipeline: host folds phi into x, wraps, and quantizes the SHIFTED phase
c8 = round((theta + pi/2)/q) to int8 (q = 2pi/256; int8 wraparound == mod
2pi).  Device per 128-row t-chunk:
  - ScalarE Sin table with scale=q on c8 -> sin(theta+pi/2) = cos(theta)
  - DVE custom even deg-6 poly in c8^2 -> cos(q*c8) = -sin(theta)
    (coefficients pre-scaled by q^2k; one 7-stage fused instruction)
  - a small head of DD batch columns gets a second stream s8 = round(theta/q)
    so ScalarE (Sin, scale=-q) also produces -sin there, balancing engines
  - TensorE contracts t against m as a [128,1] fp16 stationary into PSUM;
    both value tiles use the SAME +m stationary, so the im row accumulates
    -im; the readout flips the final sign.
End-to-end quantization error (simulated): ~7.2e-3 rel vs 2e-2 tolerance.
Data parallel over batch: core i gets columns [2048*i, 2048*(i+1)).
"""

import numpy as np

T = 2048
NUM_BLOCKS = 4
BATCH = 16384
N_CORES = 8
BPC = BATCH // N_CORES      # batch per core
KCHUNKS = T // 128          # t-chunks of 128 partitions
GB = 4                      # chunks per activation/DMA group
NGRP = KCHUNKS // GB        # groups
DD = 160                    # batch cols of -sin done on ScalarE (dual-encoded)
Q = 2.0 * np.pi / 256.0     # int8 phase quantum

# deg-6 even minimax for cos on [-pi, pi] (max err 1.4e-3)
COS6 = (9.98592512e-01, -4.95341442e-01, 3.92267876e-02, -9.69660969e-04)

_STATE = {}


def _precompute_v(weights: np.ndarray) -> np.ndarray:
    """Column 0 of the composed phasor operator, in f64."""
    wf = weights.astype(np.float64).reshape(NUM_BLOCKS, 2, T)
    c = np.zeros(T, dtype=np.complex128)
    c[0] = 1.0
    for b in range(NUM_BLOCKS - 1, -1, -1):
        c = c * np.exp(1j * wf[b, 1])
        c = np.fft.fft(c, norm="ortho")
        c = c * np.exp(1j * wf[b, 0])
    return c


def _register_cos6():
    """Register the fused even deg-6 cos polynomial as a custom DVE op.

    out = in1 + w*(s0 + w*(s1 + w*imm2)) with w = in0^2 (7 ALU stages).
    """
    import concourse.dve_ops as dve_ops
    from concourse.dve_ops import DveOp
    from concourse.dve_spec import (C0, C1, C2, C3, Spec, Src0,
                                    _spill_c3_to_src1, lower, sq)
    from concourse.dve_uop import DveOpSpec

    for op in dve_ops.OPS:
        if op.name == "COS6_ANT":
            return op

    w = sq(Src0)
    body = C3 + w * (C0 + w * (C1 + w * C2))
    spec = Spec(
        body=_spill_c3_to_src1(body),
        reference=lambda in0, in1, s0, s1, imm2: (
            in1 + (in0 * in0)
            * (s0 + (in0 * in0) * (s1 + (in0 * in0) * imm2))
        ),
    )
    name = "COS6_ANT"
    opcode = dve_ops._CUSTOM_DVE_ROW_BASE + len(dve_ops.OPS)
    shas = {}
    for ver in ("v3", "v4"):
        uops = lower(spec, ver=ver)
        shas[ver] = DveOpSpec(name=name, opcode=opcode, uops=uops,
                              rd1_en=True).sha(ver)
    op = DveOp(name, spec, subdim=False, uops_sha=shas)
    dve_ops.OPS.append(op)
    dve_ops._SUB_OPCODE_FOR_NAME[name] = opcode
    dve_ops.CUSTOM_DVE_SPECS[name] = spec
    return op


def _build_nc():
    import concourse.bacc as bacc
    import concourse.bass as bass
    import concourse.mybir as mybir
    import concourse.tile as tile

    cos6 = _register_cos6()

    i8 = mybir.dt.int8
    f16 = mybir.dt.float16
    f32 = mybir.dt.float32
    AF = mybir.ActivationFunctionType
    Alu = mybir.AluOpType

    nc = bacc.Bacc("TRN2")
    # c8[t, b] = round(wrap(theta + pi/2)/q), t-major
    c8d = nc.declare_dram_parameter("c8", [T, BPC], i8, isOutput=False)
    # s8[t, b] = round(wrap(theta)/q) for the first DD batch cols of the core
    s8d = nc.declare_dram_parameter("s8", [T, DD], i8, isOutput=False)
    mw = nc.declare_dram_parameter("mw", [128, KCHUNKS], f16, isOutput=False)
    # out[p, jj] = batch 16p + jj of this core's shard
    out = nc.declare_dram_parameter("out", [128, BPC // 128], f32, isOutput=True)

    with tile.TileContext(nc) as tc:
        with (
            tc.tile_pool(name="consts", bufs=1) as consts,
            tc.tile_pool(name="c8p", bufs=1) as c8p,
            tc.tile_pool(name="vals", bufs=3) as vp,
            tc.tile_pool(name="psum", bufs=1, space=bass.MemorySpace.PSUM) as psp,
            tc.tile_pool(name="ro", bufs=2) as rop,
        ):
            mw_t = consts.tile([128, KCHUNKS], f16)
            nc.gpsimd.dma_start(out=mw_t[:], in_=mw[:])
            c0t = consts.tile([128, 1], f32)
            nc.vector.memset(c0t, float(COS6[0]))

            # whole input resident: 4 groups x [128, GB, BPC] int8,
            # alternating HWDGE queues for fill speed
            c8t, s8t = [], []
            for j in range(NGRP):
                ct = c8p.tile([128, GB, BPC], i8, tag=f"c8_{j}", name=f"c8_{j}")
                eng = nc.sync if (j % 2 == 0) else nc.scalar
                eng.dma_start(
                    out=ct[:],
                    in_=c8d[j * GB * 128:(j + 1) * GB * 128, :].rearrange(
                        "(c p) f -> p c f", c=GB))
                c8t.append(ct)
                st = c8p.tile([128, GB, DD], i8, tag=f"s8_{j}", name=f"s8_{j}")
                nc.gpsimd.dma_start(
                    out=st[:],
                    in_=s8d[j * GB * 128:(j + 1) * GB * 128, :].rearrange(
                        "(c p) f -> p c f", c=GB))
                s8t.append(st)

            ps_im = psp.tile([1, BPC], f32, tag="im", name="ps_im")
            ps_re = psp.tile([1, BPC], f32, tag="re", name="ps_re")

            s0 = float(COS6[1] * Q * Q)
            s1 = float(COS6[2] * Q ** 4)
            imm2 = float(COS6[3] * Q ** 6)

            for g in range(NGRP):
                cosv = vp.tile([128, GB, BPC], f16, tag="cos")
                nsin = vp.tile([128, GB, BPC], f16, tag="nsin")
                # ScalarE: cos(theta) over the whole group in one pass
                nc.scalar.activation(out=cosv[:], in_=c8t[g][:],
                                     func=AF.Sin, scale=float(Q))
                # ScalarE head: -sin(theta) on the first DD cols (s8 stream)
                nc.scalar.activation(out=nsin[:, :, 0:DD], in_=s8t[g][:],
                                     func=AF.Sin, scale=float(-Q))
                # DVE: -sin(theta) = cos(q*c8) on the rest (even poly)
                nc.vector._custom_dve(
                    cos6, out=nsin[:, :, DD:BPC], in0=c8t[g][:, :, DD:BPC],
                    in1=c0t[:], s0=s0, s1=s1, imm2=imm2)
                for c in range(GB):
                    k = GB * g + c
                    first, last = (k == 0), (k == KCHUNKS - 1)
                    for j in range(BPC // 512):
                        sl = slice(j * 512, (j + 1) * 512)
                        nc.tensor.matmul(ps_im[:, sl], mw_t[:, k:k + 1],
                                         nsin[:, c, sl], start=first, stop=last)
                        nc.tensor.matmul(ps_re[:, sl], mw_t[:, k:k + 1],
                                         cosv[:, c, sl], start=first, stop=last)

            # Readout.  PSUM rows are copied to SBUF, scattered by DMA to
            # [128, 16] (partition p holds batches 16p..16p+15) so the angle
            # math runs on all 128 lanes, then:
            #   u=|im|, r=|re|, a=min/max, t0=atan(a) in [0,pi/4]
            #   angle=|g*pi/2 - t0| with g=(u>r)
            #   out = angle * (-sign(imv))   (imv holds -im)
            rowboth = rop.tile([1, 2 * BPC], f32, tag="rowboth")
            nc.vector.tensor_copy(rowboth[:, 0:BPC], ps_im[:])
            nc.scalar.copy(out=rowboth[:, BPC:2 * BPC], in_=ps_re[:])
            impp = rop.tile([128, 2, 16], f32, tag="impp")
            nc.gpsimd.dma_start(
                out=impp[:, 0, :],
                in_=rowboth[:, 0:BPC].rearrange("o (p f) -> o p f", p=128))
            nc.gpsimd.dma_start(
                out=impp[:, 1, :],
                in_=rowboth[:, BPC:2 * BPC].rearrange("o (p f) -> o p f", p=128))
            imv = impp[:, 0, :]
            rev = impp[:, 1, :]
            u = rop.tile([128, 16], f32, tag="u")
            nc.scalar.activation(out=u[:], in_=imv, func=AF.Abs)
            r = rop.tile([128, 16], f32, tag="r")
            nc.scalar.activation(out=r[:], in_=rev, func=AF.Abs)
            sgn = rop.tile([128, 16], f32, tag="sgn")
            nc.scalar.sign(out=sgn[:], in_=imv)
            mn = rop.tile([128, 16], f32, tag="mn")
            nc.vector.tensor_tensor(mn[:], u[:], r[:], Alu.min)
            mx = rop.tile([128, 16], f32, tag="mx")
            nc.vector.tensor_tensor(mx[:], u[:], r[:], Alu.max)
            rc = rop.tile([128, 16], f32, tag="rc")
            nc.vector.reciprocal(out=rc[:], in_=mx[:])
            aq = rop.tile([128, 16], f32, tag="aq")
            nc.vector.tensor_mul(aq[:], mn[:], rc[:])
            g8 = rop.tile([128, 16], f32, tag="g8")
            nc.vector.tensor_tensor(g8[:], u[:], r[:], Alu.is_gt)
            t0 = rop.tile([128, 16], f32, tag="t0")
            nc.scalar.activation(out=t0[:], in_=aq[:], func=AF.Arctan)
            d = rop.tile([128, 16], f32, tag="d")
            nc.vector.scalar_tensor_tensor(
                out=d[:], in0=g8[:], scalar=float(np.pi / 2), in1=t0[:],
                op0=Alu.mult, op1=Alu.subtract)
            angle = rop.tile([128, 16], f32, tag="angle")
            nc.vector.scalar_tensor_tensor(
                out=angle[:], in0=d[:], scalar=-1.0, in1=d[:],
                op0=Alu.mult, op1=Alu.max)
            o = rop.tile([128, 16], f32, tag="o")
            # imv holds -im, so flip the sign: out = angle * (-sgn)
            nc.vector.scalar_tensor_tensor(
                out=o[:], in0=sgn[:], scalar=-1.0, in1=angle[:],
                op0=Alu.mult, op1=Alu.mult)
            nc.gpsimd.dma_start(out=out[:], in_=o[:])

    nc.compile()
    return nc


def _enc_int8(a: np.ndarray) -> np.ndarray:
    """round(wrap(a)/q) as int8 with 128 -> -128 (same angle mod 2pi)."""
    w = (a + np.float32(np.pi)) % np.float32(2 * np.pi) - np.float32(np.pi)
    n = np.rint(w * np.float32(1.0 / Q))
    n = np.where(n >= 128, n - 256, n)
    return n.astype(np.int8)


def _prepare_inputs(x: np.ndarray, weights: np.ndarray):
    v = _precompute_v(np.asarray(weights))
    m = np.abs(v).astype(np.float32)
    phi = np.angle(v).astype(np.float32)

    theta = np.asarray(x, dtype=np.float32) + phi[None, :]   # [B, T]
    c8 = _enc_int8(theta + np.float32(np.pi / 2))
    # s8 only needed for the first DD local columns of each core
    mw = np.ascontiguousarray(m.reshape(KCHUNKS, 128).T).astype(np.float16)

    in_maps = []
    for i in range(N_CORES):
        sl = slice(i * BPC, (i + 1) * BPC)
        c8s = np.ascontiguousarray(c8[sl].T)                  # [T, BPC]
        s8s = np.ascontiguousarray(
            _enc_int8(theta[i * BPC:i * BPC + DD]).T)         # [T, DD]
        in_maps.append({"c8": c8s, "s8": s8s, "mw": mw})
    return in_maps


def _run(x: np.ndarray, weights: np.ndarray, trace: bool = False):
    from concourse.bass_utils import run_bass_kernel_spmd

    if "nc" not in _STATE:
        _STATE["nc"] = _build_nc()
    nc = _STATE["nc"]

    in_maps = _prepare_inputs(x, weights)
    res = run_bass_kernel_spmd(nc, in_maps, list(range(N_CORES)), trace=trace)
    out = np.concatenate(
        [res.results[i]["out"].reshape(BPC) for i in range(N_CORES)]
    ).astype(np.float32)
    return out, res


def kernel(x: np.ndarray, weights: np.ndarray) -> np.ndarray:
    out, _ = _run(np.asarray(x), np.asarray(weights))
    return out


# revision 3
# speedup vs baseline: 1.0888x; 1.0888x over previous
"""PhasorTransformer kernel for 8x TRN2 NeuronCores (v3).

Math: the reference applies, per batch row b, 4 blocks of
(diag phase shift -> ortho DFT -> diag phase shift) to z0 = exp(i*x[b,:]),
then reads out asin(sin(angle(z[:, 0]))).  Everything after z0 is linear in
z0, so z_final[b, 0] = <z0[b, :], v> for a fixed complex vector v ("column 0"
of the composed operator) that depends only on the weights.  With
v[t] = m[t] * exp(i*phi[t]):

    re[b] = sum_t m[t] * cos(x[b,t] + phi[t])
    im[b] = sum_t m[t] * sin(x[b,t] + phi[t])
    out[b] = atan-fold(im / |re|) * sign(im)

Host folds phi into x, wraps, and quantizes the SHIFTED phase
c8 = round((theta + pi/2)/q) to int8 (q = 2pi/256; int8 wraparound == mod
2pi).  Device, per t-chunk of 128 partitions:
  - ScalarE Sin table (scale=q) on c8 -> sin(theta+pi/2) = cos(theta)
  - DVE custom even deg-6 poly in c8^2 -> cos(q*c8) = -sin(theta)
    (coefficients pre-scaled by q^2k; one 7-stage fused instruction)
  - a head of DD batch cols gets a second stream s8 = round(theta/q) so
    ScalarE (Sin, scale=-q) also yields -sin there, balancing the engines
  - TensorE contracts t against m ([128,1] fp16 stationary) into PSUM;
    both value tiles share the +m stationary so the im row holds -im and
    the readout flips the sign bit.
Readout runs entirely on the DVE (bit-trick abs/sign, approx reciprocal,
odd deg-7 atan custom op) to avoid cross-engine hops and table loads.
Chunk groups are sized [2,2,4,4,4] and each group's int8 tile is fetched
as two half-DMAs on the two HWDGE rings (sync/scalar) so group 0 lands
first and the eval pipeline starts early.
End-to-end quantization error (simulated): ~7.2e-3 rel vs 2e-2 tolerance.
Data parallel over batch: core i gets columns [2048*i, 2048*(i+1)).
"""

import numpy as np

T = 2048
NUM_BLOCKS = 4
BATCH = 16384
N_CORES = 8
BPC = BATCH // N_CORES      # batch per core
KCHUNKS = T // 128          # t-chunks of 128 partitions
GROUPS = ((0, 2), (2, 2), (4, 4), (8, 4), (12, 4))  # (start chunk, n chunks)
DD = 144                    # batch cols of -sin done on ScalarE (dual stream)
Q = 2.0 * np.pi / 256.0     # int8 phase quantum

# deg-6 even minimax for cos on [-pi, pi] (max err 1.4e-3)
COS6 = (9.98592512e-01, -4.95341442e-01, 3.92267876e-02, -9.69660969e-04)
# deg-7 odd minimax for atan on [0, 1] (max err 8.2e-5)
ATAN7 = (9.9921454e-01, -3.2118204e-01, 1.4628138e-01, -3.899779e-02)

_STATE = {}


def _precompute_v(weights: np.ndarray) -> np.ndarray:
    """Column 0 of the composed phasor operator, in f64."""
    wf = weights.astype(np.float64).reshape(NUM_BLOCKS, 2, T)
    c = np.zeros(T, dtype=np.complex128)
    c[0] = 1.0
    for b in range(NUM_BLOCKS - 1, -1, -1):
        c = c * np.exp(1j * wf[b, 1])
        c = np.fft.fft(c, norm="ortho")
        c = c * np.exp(1j * wf[b, 0])
    return c


def _register_ops():
    """Register COS6_ANT (even deg-6 poly) and ODD7_ANT (odd deg-7 poly)."""
    import concourse.dve_ops as dve_ops
    from concourse.dve_ops import DveOp
    from concourse.dve_spec import (C0, C1, C2, C3, Spec, Src0,
                                    _spill_c3_to_src1, lower, sq)
    from concourse.dve_uop import DveOpSpec

    have = {op.name: op for op in dve_ops.OPS}
    out = []
    w = sq(Src0)
    specs = {
        # out = in1 + w*(s0 + w*(s1 + w*imm2)), w = in0^2
        "COS6_ANT": Spec(
            body=_spill_c3_to_src1(C3 + w * (C0 + w * (C1 + w * C2))),
            reference=lambda in0, in1, s0, s1, imm2: (
                in1 + (in0 * in0)
                * (s0 + (in0 * in0) * (s1 + (in0 * in0) * imm2))
            ),
        ),
        # out = in0*(in1 + w*(s0 + w*(s1 + w*imm2))), w = in0^2
        "ODD7_ANT": Spec(
            body=_spill_c3_to_src1(Src0 * (C3 + w * (C0 + w * (C1 + w * C2)))),
            reference=lambda in0, in1, s0, s1, imm2: (
                in0 * (in1 + (in0 * in0)
                       * (s0 + (in0 * in0) * (s1 + (in0 * in0) * imm2)))
            ),
        ),
    }
    for name, spec in specs.items():
        if name in have:
            out.append(have[name])
            continue
        opcode = dve_ops._CUSTOM_DVE_ROW_BASE + len(dve_ops.OPS)
        shas = {}
        for ver in ("v3", "v4"):
            uops = lower(spec, ver=ver)
            shas[ver] = DveOpSpec(name=name, opcode=opcode, uops=uops,
                                  rd1_en=True).sha(ver)
        op = DveOp(name, spec, subdim=False, uops_sha=shas)
        dve_ops.OPS.append(op)
        dve_ops._SUB_OPCODE_FOR_NAME[name] = opcode
        dve_ops.CUSTOM_DVE_SPECS[name] = spec
        out.append(op)
    return out


def _build_nc():
    import concourse.bacc as bacc
    import concourse.bass as bass
    import concourse.mybir as mybir
    import concourse.tile as tile
    from concourse.dve_ops import RECIP_APPROX_FAST_CONSTS, RECIPROCAL_APPROX_FAST

    cos6, odd7 = _register_ops()

    i8 = mybir.dt.int8
    u32 = mybir.dt.uint32
    f16 = mybir.dt.float16
    f32 = mybir.dt.float32
    AF = mybir.ActivationFunctionType
    Alu = mybir.AluOpType

    nc = bacc.Bacc("TRN2")
    # c8[t, b] = round(wrap(theta + pi/2)/q), t-major
    c8d = nc.declare_dram_parameter("c8", [T, BPC], i8, isOutput=False)
    # s8[t, b] = round(wrap(theta)/q) for the first DD batch cols of the core
    s8d = nc.declare_dram_parameter("s8", [T, DD], i8, isOutput=False)
    mw = nc.declare_dram_parameter("mw", [128, KCHUNKS], f16, isOutput=False)
    # out[p, jj] = batch 16p + jj of this core's shard
    out = nc.declare_dram_parameter("out", [128, BPC // 128], f32, isOutput=True)

    with tile.TileContext(nc) as tc:
        with (
            tc.tile_pool(name="consts", bufs=1) as consts,
            tc.tile_pool(name="c8p", bufs=1) as c8p,
            tc.tile_pool(name="vals", bufs=1) as vp,
            tc.tile_pool(name="psum", bufs=1, space=bass.MemorySpace.PSUM) as psp,
            tc.tile_pool(name="ro", bufs=1) as rop,
        ):
            mw_t = consts.tile([128, KCHUNKS], f16)
            nc.gpsimd.dma_start(out=mw_t[:], in_=mw[:])
            c0t = consts.tile([128, 1], f32)
            nc.vector.memset(c0t, float(COS6[0]))
            a0t = consts.tile([128, 1], f32)
            nc.vector.memset(a0t, float(ATAN7[0]))

            # per-group input tiles; each filled by two half-DMAs, one per
            # HWDGE ring, issued in group order so early groups land first
            c8t, s8t = [], []
            for gi, (k0, n) in enumerate(GROUPS):
                ct = c8p.tile([128, n, BPC], i8, tag=f"c8_{gi}", name=f"c8_{gi}")
                h = n // 2
                for half, eng in ((0, nc.sync), (1, nc.scalar)):
                    lo = k0 + half * h
                    eng.dma_start(
                        out=ct[:, half * h:(half + 1) * h, :],
                        in_=c8d[lo * 128:(lo + h) * 128, :].rearrange(
                            "(c p) f -> p c f", c=h))
                c8t.append(ct)
                st = c8p.tile([128, n, DD], i8, tag=f"s8_{gi}", name=f"s8_{gi}")
                nc.gpsimd.dma_start(
                    out=st[:],
                    in_=s8d[k0 * 128:(k0 + n) * 128, :].rearrange(
                        "(c p) f -> p c f", c=n))
                s8t.append(st)

            ps_im = psp.tile([1, BPC], f32, tag="im", name="ps_im")
            ps_re = psp.tile([1, BPC], f32, tag="re", name="ps_re")

            s0 = float(COS6[1] * Q * Q)
            s1 = float(COS6[2] * Q ** 4)
            imm2 = float(COS6[3] * Q ** 6)

            for gi, (k0, n) in enumerate(GROUPS):
                cosv = vp.tile([128, n, BPC], f16, tag=f"cos{gi}")
                nsin = vp.tile([128, n, BPC], f16, tag=f"nsin{gi}")
                # ScalarE: cos(theta) for the whole group in one pass
                nc.scalar.activation(out=cosv[:], in_=c8t[gi][:],
                                     func=AF.Sin, scale=float(Q))
                # ScalarE head: -sin(theta) on the first DD cols (s8 stream)
                nc.scalar.activation(out=nsin[:, :, 0:DD], in_=s8t[gi][:],
                                     func=AF.Sin, scale=float(-Q))
                # DVE: -sin(theta) = cos(q*c8) on the rest (even poly)
                nc.vector._custom_dve(
                    cos6, out=nsin[:, :, DD:BPC], in0=c8t[gi][:, :, DD:BPC],
                    in1=c0t[:], s0=s0, s1=s1, imm2=imm2)
                for c in range(n):
                    k = k0 + c
                    first, last = (k == 0), (k == KCHUNKS - 1)
                    for j in range(BPC // 512):
                        sl = slice(j * 512, (j + 1) * 512)
                        nc.tensor.matmul(ps_im[:, sl], mw_t[:, k:k + 1],
                                         nsin[:, c, sl], start=first, stop=last)
                        nc.tensor.matmul(ps_re[:, sl], mw_t[:, k:k + 1],
                                         cosv[:, c, sl], start=first, stop=last)

            # Readout.  PSUM rows -> SBUF (DVE im / ScalarE re in parallel),
            # DMA-scatter to [128, 2, 16] (partition p holds batches
            # 16p..16p+15), then a pure-DVE chain:
            #   u=|im|, r=|re| (bit and), a=min/max, rc~=1/max (approx),
            #   t0=atan7(a) in [0,pi/4], angle=|g*pi/2 - t0| with g=(u>r),
            #   out = angle with sign bit of -imv  (imv holds -im)
            rowboth = rop.tile([1, 2 * BPC], f32, tag="rowboth")
            nc.vector.tensor_copy(rowboth[:, 0:BPC], ps_im[:])
            nc.scalar.copy(out=rowboth[:, BPC:2 * BPC], in_=ps_re[:])
            impp = rop.tile([128, 2, 16], f32, tag="impp")
            nc.sync.dma_start(
                out=impp[:, 0, :],
                in_=rowboth[:, 0:BPC].rearrange("o (p f) -> o p f", p=128))
            nc.sync.dma_start(
                out=impp[:, 1, :],
                in_=rowboth[:, BPC:2 * BPC].rearrange("o (p f) -> o p f", p=128))
            imv = impp[:, 0, :]
            ur = rop.tile([128, 2, 16], f32, tag="ur")
            nc.vector.tensor_scalar(
                out=ur[:].bitcast(u32), in0=impp[:].bitcast(u32),
                scalar1=0x7FFFFFFF, scalar2=None, op0=Alu.bitwise_and)
            u = ur[:, 0, :]
            r = ur[:, 1, :]
            mn = rop.tile([128, 16], f32, tag="mn")
            nc.vector.tensor_tensor(mn[:], u, r, Alu.min)
            mx = rop.tile([128, 16], f32, tag="mx")
            nc.vector.tensor_tensor(mx[:], u, r, Alu.max)
            g8 = rop.tile([128, 16], f32, tag="g8")
            nc.vector.tensor_tensor(g8[:], u, r, Alu.is_gt)
            sb = rop.tile([128, 16], f32, tag="sb")
            nc.vector.tensor_scalar(
                out=sb[:].bitcast(u32), in0=imv.bitcast(u32),
                scalar1=0x80000000, scalar2=0x80000000,
                op0=Alu.bitwise_xor, op1=Alu.bitwise_and)
            rc = rop.tile([128, 16], f32, tag="rc")
            nc.vector._custom_dve(
                RECIPROCAL_APPROX_FAST, out=rc[:], in0=mx[:],
                **RECIP_APPROX_FAST_CONSTS)
            aq = rop.tile([128, 16], f32, tag="aq")
            nc.vector.tensor_mul(aq[:], mn[:], rc[:])
            t0 = rop.tile([128, 16], f32, tag="t0")
            nc.vector._custom_dve(
                odd7, out=t0[:], in0=aq[:], in1=a0t[:],
                s0=float(ATAN7[1]), s1=float(ATAN7[2]), imm2=float(ATAN7[3]))
            d = rop.tile([128, 16], f32, tag="d")
            nc.vector.scalar_tensor_tensor(
                out=d[:], in0=g8[:], scalar=float(np.pi / 2), in1=t0[:],
                op0=Alu.mult, op1=Alu.subtract)
            angle = rop.tile([128, 16], f32, tag="angle")
            nc.vector.tensor_scalar(
                out=angle[:].bitcast(u32), in0=d[:].bitcast(u32),
                scalar1=0x7FFFFFFF, scalar2=None, op0=Alu.bitwise_and)
            o = rop.tile([128, 16], f32, tag="o")
            nc.vector.tensor_tensor(
                o[:].bitcast(u32), angle[:].bitcast(u32), sb[:].bitcast(u32),
                Alu.bitwise_or)
            nc.sync.dma_start(out=out[:], in_=o[:])

    nc.compile()
    return nc


def _enc_int8(a: np.ndarray) -> np.ndarray:
    """round(wrap(a)/q) as int8 with 128 -> -128 (same angle mod 2pi)."""
    w = (a + np.float32(np.pi)) % np.float32(2 * np.pi) - np.float32(np.pi)
    n = np.rint(w * np.float32(1.0 / Q))
    n = np.where(n >= 128, n - 256, n)
    return n.astype(np.int8)


def _prepare_inputs(x: np.ndarray, weights: np.ndarray):
    v = _precompute_v(np.asarray(weights))
    m = np.abs(v).astype(np.float32)
    phi = np.angle(v).astype(np.float32)

    theta = np.asarray(x, dtype=np.float32) + phi[None, :]   # [B, T]
    c8 = _enc_int8(theta + np.float32(np.pi / 2))
    mw = np.ascontiguousarray(m.reshape(KCHUNKS, 128).T).astype(np.float16)

    in_maps = []
    for i in range(N_CORES):
        sl = slice(i * BPC, (i + 1) * BPC)
        c8s = np.ascontiguousarray(c8[sl].T)                  # [T, BPC]
        s8s = np.ascontiguousarray(
            _enc_int8(theta[i * BPC:i * BPC + DD]).T)         # [T, DD]
        in_maps.append({"c8": c8s, "s8": s8s, "mw": mw})
    return in_maps


def _run(x: np.ndarray, weights: np.ndarray, trace: bool = False):
    from concourse.bass_utils import run_bass_kernel_spmd

    if "nc" not in _STATE:
        _STATE["nc"] = _build_nc()
    nc = _STATE["nc"]

    in_maps = _prepare_inputs(x, weights)
    res = run_bass_kernel_spmd(nc, in_maps, list(range(N_CORES)), trace=trace)
    out = np.concatenate(
        [res.results[i]["out"].reshape(BPC) for i in range(N_CORES)]
    ).astype(np.float32)
    return out, res


def kernel(x: np.ndarray, weights: np.ndarray) -> np.ndarray:
    out, _ = _run(np.asarray(x), np.asarray(weights))
    return out
